# revision 4
# baseline (speedup 1.0000x reference)
"""Trainium2 Bass kernel for the DVR-JANET recurrent cell.

Strategy (per spec sharding hint): data-parallel over batch across 8 cores
(8 sequences each), weights replicated.  Per core the T=1024 sequential
recurrence runs fully unrolled with all tensors in a transposed layout
[h on partitions, batch on free], so the 7 HxH matmuls per step use the
weights as PE-stationary operands (fp16, fast weight load) and the
activations as tiny moving operands.  sin/cos and all tanh-family gates are
evaluated by the Scalar engine out of one pinned activation-table set
(silu_and_others: sin + tanh) to avoid per-step table swaps; sigmoid is
rewritten as tanh.  Rank-1 input terms and all biases are folded into two
tiny block-diagonal matmuls.  The per-step elementwise chain is 5 fused
Vector-engine ops.  Final projections to I/Q run as a batched matmul pass
over the fp16 state history kept resident in SBUF.
"""

import functools
import numpy as np

import concourse.bacc as bacc
import concourse.mybir as mybir
from concourse import tile
import concourse.hw_specs as hw_specs
from concourse.bass_utils import run_bass_kernel_spmd

F32 = mybir.dt.float32
F16 = mybir.dt.float16
AF = mybir.ActivationFunctionType
OP = mybir.AluOpType

B, T, H = 64, 1024, 256
NCORES = 8
BL = B // NCORES          # batch per core = 8
CH = 128                  # XB chunk length (steps)
NCH = T // CH
NT = 32                   # full weight tiles

# ---------------------------------------------------------------------------
# Pin the ACT table set to silu_and_others (contains sin AND tanh) so the
# compiler never inserts per-step table swaps.
_orig_tables = hw_specs.get_activation_tables.__wrapped__


def _pinned_tables(arch):
    full = _orig_tables(arch)
    return {name: (funcs if name == "silu_and_others" else set())
            for name, funcs in full.items()}


def _pin_tables():
    fn = functools.cache(_pinned_tables)
    hw_specs.get_activation_tables = fn
    if hasattr(bacc, "get_activation_tables"):
        bacc.get_activation_tables = fn


# ---------------------------------------------------------------------------
_PROG_CACHE = {}


def build_program(Tn=T, sb=0.0):
    """Build the full 8-core SPMD program (same program, per-core inputs)."""
    key = (Tn, float(sb))
    if key in _PROG_CACHE:
        return _PROG_CACHE[key]
    _pin_tables()
    nch = max(1, (Tn + CH - 1) // CH)
    nc = bacc.Bacc("TRN2", target_bir_lowering=False, debug=False,
                   num_devices=NCORES)

    # DRAM I/O (per core)
    w1_d = nc.dram_tensor("W1", [128, NT * 128], F16, kind="ExternalInput").ap()
    xwa_d = nc.dram_tensor("XWA", [5, 128], F16, kind="ExternalInput").ap()
    xwb_d = nc.dram_tensor("XWB", [2, 128], F16, kind="ExternalInput").ap()
    xwg_d = nc.dram_tensor("XWG", [6, 128], F16, kind="ExternalInput").ap()
    onesg_d = nc.dram_tensor("ONESG", [6, 48], F16, kind="ExternalInput").ap()
    xba_d = nc.dram_tensor("XBA", [nch, 5, CH * 32], F16, kind="ExternalInput").ap()
    xbb_d = nc.dram_tensor("XBB", [nch, 2, CH * 16], F16, kind="ExternalInput").ap()
    wp_d = nc.dram_tensor("WP", [128, 4], F16, kind="ExternalInput").ap()
    s0_d = nc.dram_tensor("S0", [128, 32], F16, kind="ExternalInput").ap()
    out_d = nc.dram_tensor("OUT", [1, 2 * Tn * BL], F32, kind="ExternalOutput").ap()

    with tile.TileContext(nc, trace_sim=False) as tc:
        with (
            tc.tile_pool(name="const", bufs=1) as cpool,
            tc.tile_pool(name="buf", bufs=1) as bufpool,
            tc.tile_pool(name="xba", bufs=2) as xbapool,
            tc.tile_pool(name="xbb", bufs=2) as xbbpool,
            tc.tile_pool(name="work", bufs=3) as wpool,
            tc.tile_pool(name="pa", bufs=2, space="PSUM") as ppa,
            tc.tile_pool(name="pb", bufs=2, space="PSUM") as ppb,
            tc.tile_pool(name="pg", bufs=2, space="PSUM") as ppg,
            tc.tile_pool(name="pp", bufs=2, space="PSUM") as ppp,
        ):
            wt1 = cpool.tile([128, NT * 128], F16, tag="wt1")
            xwa = cpool.tile([5, 128], F16, tag="xwa")
            xwb = cpool.tile([2, 128], F16, tag="xwb")
            xwg = cpool.tile([6, 128], F16, tag="xwg")
            onesg = cpool.tile([6, 48], F16, tag="onesg")
            wp = cpool.tile([128, 4], F16, tag="wp")
            # +2 slots: slot 0 = h0; slot t+1 = state after step t; one pad
            # slot so the projection's strided rhs windows stay in-bounds.
            buf = bufpool.tile([128, 32 * (Tn + 2)], F16, tag="buf")

            nc.sync.dma_start(wt1[:], w1_d)
            nc.sync.dma_start(xwa[:], xwa_d)
            nc.sync.dma_start(xwb[:], xwb_d)
            nc.sync.dma_start(xwg[:], xwg_d)
            nc.sync.dma_start(onesg[:], onesg_d)
            nc.sync.dma_start(wp[:], wp_d)
            nc.sync.dma_start(buf[:, 0:32], s0_d)

            def wtile(i):
                return wt1[:, 128 * i:128 * (i + 1)]

            xba_t = None
            xbb_t = None
            for t in range(Tn):
                s = t % CH
                if s == 0:
                    cc = t // CH
                    xba_t = xbapool.tile([5, CH * 32], F16, tag="xba")
                    xbb_t = xbbpool.tile([2, CH * 16], F16, tag="xbb")
                    nc.sync.dma_start(xba_t[:], xba_d[cc])
                    nc.sync.dma_start(xbb_t[:], xbb_d[cc])

                slot = buf[:, 32 * t:32 * (t + 1)]         # state entering step t
                nslot = buf[:, 32 * (t + 1):32 * (t + 2)]  # state after step t

                pa = ppa.tile([128, 32], F32, tag="pa")
                pb = ppb.tile([128, 16], F32, tag="pb")
                pg = ppg.tile([128, 48], F32, tag="pg")

                # --- gate bank pG: biases, then h-top contractions ---------
                nc.tensor.matmul(pg[:], xwg[:], onesg[:], start=True, stop=False)
                for j in (0, 1):
                    for k in (0, 1):
                        nc.tensor.matmul(pg[:, 8 * j:8 * j + 8],
                                         wtile(16 + 2 * j + k),
                                         slot[:, 8 * k:8 * k + 8],
                                         start=False, stop=False)
                        nc.tensor.matmul(pg[:, 16 + 8 * j:16 + 8 * j + 8],
                                         wtile(20 + 2 * j + k),
                                         slot[:, 16 + 8 * k:16 + 8 * k + 8],
                                         start=False, stop=False)

                # --- p = hI*hQ (fp16 for the PE) ---------------------------
                pt16 = wpool.tile([128, 16], F16, tag="pt16")
                nc.vector.tensor_mul(pt16[:], slot[:, 0:16], slot[:, 16:32])

                # --- theta / theta' bank pA --------------------------------
                nc.tensor.matmul(pa[:], xwa[:],
                                 xba_t[:, 32 * s:32 * s + 32],
                                 start=True, stop=False)
                for j in (0, 1):
                    for k in (0, 1):
                        nc.tensor.matmul(pa[:, 8 * j:8 * j + 8],
                                         wtile(2 * j + k),
                                         pt16[:, 8 * k:8 * k + 8],
                                         start=False, stop=(k == 1))
                        nc.tensor.matmul(pa[:, 16 + 8 * j:16 + 8 * j + 8],
                                         wtile(4 + 2 * j + k),
                                         pt16[:, 8 * k:8 * k + 8],
                                         start=False, stop=(k == 1))

                # --- a bank pB ---------------------------------------------
                nc.tensor.matmul(pb[:], xwb[:],
                                 xbb_t[:, 16 * s:16 * s + 16],
                                 start=True, stop=False)
                for j in (0, 1):
                    for k in (0, 1):
                        nc.tensor.matmul(pb[:, 8 * j:8 * j + 8],
                                         wtile(8 + 2 * j + k),
                                         pt16[:, 8 * k:8 * k + 8],
                                         start=False, stop=(k == 1))

                # --- f' into pG --------------------------------------------
                for j in (0, 1):
                    for k in (0, 1):
                        nc.tensor.matmul(pg[:, 32 + 8 * j:32 + 8 * j + 8],
                                         wtile(12 + 2 * j + k),
                                         pt16[:, 8 * k:8 * k + 8],
                                         start=False, stop=(k == 1))

                # --- SC = sin([theta | theta+pi/2]) = [sin | cos] ----------
                sct = wpool.tile([128, 32], F32, tag="sct")
                nc.scalar.activation(sct[:], pa[:], AF.Sin)

                # --- casa = (a + sb) * SC  -> [sa | ca] fp16 ---------------
                casa = wpool.tile([128, 32], F16, tag="casa")
                a_b = pb[:].rearrange("p (o f) -> p o f", o=1).broadcast_to([128, 2, 16])
                nc.vector.scalar_tensor_tensor(casa[:], a_b, float(sb), sct[:],
                                               OP.add, OP.mult)

                # --- bottom contractions into pG ---------------------------
                for j in (0, 1):
                    for k in (0, 1):
                        nc.tensor.matmul(pg[:, 8 * j:8 * j + 8],
                                         wtile(24 + 2 * j + k),
                                         casa[:, 16 + 8 * k:16 + 8 * k + 8],
                                         start=False, stop=(k == 1))
                        nc.tensor.matmul(pg[:, 16 + 8 * j:16 + 8 * j + 8],
                                         wtile(28 + 2 * j + k),
                                         casa[:, 8 * k:8 * k + 8],
                                         start=False, stop=(k == 1))

                # --- TH = tanh([gc | gs | f']) -----------------------------
                gbf = wpool.tile([128, 48], F32, tag="gbf")
                nc.scalar.activation(gbf[:], pg[:], AF.Tanh)

                # --- state update -----------------------------------------
                # S_new = 0.5*(1+tau)*(S - g) + g
                d = wpool.tile([128, 32], F32, tag="d")
                m2 = wpool.tile([128, 32], F32, tag="m2")
                nc.vector.tensor_tensor(d[:], slot[:], gbf[:, 0:32], OP.subtract)
                tau_b = gbf[:, 32:48].rearrange("p (o f) -> p o f", o=1) \
                                     .broadcast_to([128, 2, 16])
                nc.vector.scalar_tensor_tensor(m2[:], tau_b, 1.0, d[:],
                                               OP.add, OP.mult)
                nc.vector.scalar_tensor_tensor(nslot, m2[:], 0.5, gbf[:, 0:32],
                                               OP.mult, OP.add)

            # ----- projection: I/Q = WI.hI / WQ.hQ over all t --------------
            iqs = cpool.tile([1, 2 * Tn * BL], F32, tag="iqs")
            nchunk = (Tn + 63) // 64
            for c in range(nchunk):
                tc0 = 64 * c
                tlen = min(64, Tn - tc0)
                for q in (0, 1):
                    pp = ppp.tile([1, 512], F32, tag="pp")
                    for j in (0, 1):
                        st = 32 * (tc0 + 1) + 16 * q + 8 * j
                        rhs = buf[:, st:st + 32 * tlen] \
                            .rearrange("p (t b) -> p t b", t=tlen)[:, :, 0:8]
                        # rhs free AP: tlen steps of 32 cols, 8 batch cols
                        nc.tensor.matmul(pp[:, 0:8 * tlen], wp[:, 2 * q + j:2 * q + j + 1],
                                         rhs, start=(j == 0), stop=(j == 1))
                    dst = iqs[0:1, q * Tn * BL + BL * tc0:
                              q * Tn * BL + BL * (tc0 + tlen)]
                    if (c + q) % 2 == 0:
                        nc.scalar.copy(dst, pp[:, 0:8 * tlen])
                    else:
                        nc.vector.tensor_copy(dst, pp[:, 0:8 * tlen])
            nc.sync.dma_start(out_d, iqs[:])

    nc.compile()
    _PROG_CACHE[key] = nc
    return nc


# ---------------------------------------------------------------------------
def prepare_inputs(inputs, Tn=T):
    """Host-side preprocessing: weight packing + per-core input maps."""
    f16 = np.float16
    x = np.asarray(inputs["x"], np.float32)
    hI0 = np.asarray(inputs["hI_0"], np.float32)[0]
    hQ0 = np.asarray(inputs["hQ_0"], np.float32)[0]
    c1 = float(np.asarray(inputs["c1"])[0])
    c2 = float(np.asarray(inputs["c2"])[0])
    c3 = float(np.asarray(inputs["c3"])[0])
    sc = c1 + c2 + c3
    sb = -(c1 / 3.0 + 2.0 * c2 / 3.0 + c3)
    Wa = np.asarray(inputs["Wa"], np.float32)[0]
    Wah = np.asarray(inputs["Wah"], np.float32)
    Wp1 = np.asarray(inputs["Wp1"], np.float32)[0]
    Wph = np.asarray(inputs["Wph"], np.float32)
    Wf = np.asarray(inputs["Wf"], np.float32)
    bf = np.asarray(inputs["bf"], np.float32)
    Wgc = np.asarray(inputs["Wgc"], np.float32)
    bgc = np.asarray(inputs["bgc"], np.float32)
    Wgs = np.asarray(inputs["Wgs"], np.float32)
    bgs = np.asarray(inputs["bgs"], np.float32)
    WI = np.asarray(inputs["WI"], np.float32)
    WQ = np.asarray(inputs["WQ"], np.float32)

    def tiles4(W):
        return [W[128 * k:128 * (k + 1), 128 * j:128 * (j + 1)]
                for j in (0, 1) for k in (0, 1)]

    tl = []
    tl += tiles4(Wph)               # 0-3 theta
    tl += tiles4(Wph)               # 4-7 theta'
    tl += tiles4(sc * Wah)          # 8-11 a (pre-scaled)
    tl += tiles4(0.5 * Wf)          # 12-15 f'
    tl += tiles4(Wgc[:H])           # 16-19 gc top
    tl += tiles4(Wgs[:H])           # 20-23 gs top
    tl += tiles4(Wgc[H:])           # 24-27 gc bot (ca)
    tl += tiles4(Wgs[H:])           # 28-31 gs bot (sa)
    W1 = np.concatenate(tl, axis=1).astype(f16)

    XWA = np.stack([Wp1[0:128], Wp1[128:256], Wp1[0:128], Wp1[128:256],
                    np.ones(128, np.float32)]).astype(f16)
    XWB = np.stack([(sc * Wa)[0:128], (sc * Wa)[128:256]]).astype(f16)
    XWG = np.stack([bgc[0:128], bgc[128:256], bgs[0:128], bgs[128:256],
                    0.5 * bf[0:128], 0.5 * bf[128:256]]).astype(f16)
    ONESG = np.zeros((6, 48), np.float32)
    for i in range(6):
        ONESG[i, 8 * i:8 * i + 8] = 1.0
    ONESG = ONESG.astype(f16)
    WP = np.stack([WI[0:128], WI[128:256], WQ[0:128], WQ[128:256]], axis=1).astype(f16)

    nch = max(1, (Tn + CH - 1) // CH)
    in_maps = []
    for c in range(NCORES):
        bs = slice(BL * c, BL * (c + 1))
        x0 = x[bs, :Tn, 0]            # [BL, Tn]
        x1 = x[bs, :Tn, 1]
        XBA = np.zeros((nch, 5, CH, 32), np.float32)
        XBB = np.zeros((nch, 2, CH, 16), np.float32)
        x1p = np.zeros((nch, CH, BL), np.float32)
        x0p = np.zeros((nch, CH, BL), np.float32)
        x1f = x1.T.reshape(-1, BL)    # [Tn, BL]
        x0f = x0.T.reshape(-1, BL)
        x1p.reshape(-1, BL)[:Tn] = x1f
        x0p.reshape(-1, BL)[:Tn] = x0f
        for r, cols in ((0, slice(0, 8)), (1, slice(8, 16)),
                        (2, slice(16, 24)), (3, slice(24, 32))):
            XBA[:, r, :, cols] = x1p
        XBA[:, 4, :, 16:32] = np.pi / 2
        XBB[:, 0, :, 0:8] = x0p
        XBB[:, 1, :, 8:16] = x0p
        S0 = np.zeros((128, 32), np.float32)
        for j in (0, 1):
            S0[:, 8 * j:8 * j + 8] = hI0[bs, 128 * j:128 * (j + 1)].T
            S0[:, 16 + 8 * j:24 + 8 * j] = hQ0[bs, 128 * j:128 * (j + 1)].T
        in_maps.append({
            "W1": W1, "XWA": XWA, "XWB": XWB, "XWG": XWG, "ONESG": ONESG,
            "WP": WP, "S0": S0.astype(f16),
            "XBA": XBA.reshape(nch, 5, CH * 32).astype(f16),
            "XBB": XBB.reshape(nch, 2, CH * 16).astype(f16),
        })
    return in_maps, sb


def assemble(results, inputs, Tn=T):
    bI = float(np.asarray(inputs["bI"])[0])
    bQ = float(np.asarray(inputs["bQ"])[0])
    out = np.zeros((B, Tn, 2), np.float32)
    nchunk = (Tn + 63) // 64
    for c in range(NCORES):
        arr = results[c]["OUT"].reshape(2, Tn * BL)
        for q in (0, 1):
            v = arr[q].reshape(nchunk, -1, BL)     # [chunk, t_in, b]
            v = v.transpose(2, 0, 1).reshape(BL, Tn)
            out[BL * c:BL * (c + 1), :, q] = v + (bI if q == 0 else bQ)
    return out


def kernel(**inputs) -> np.ndarray:
    in_maps, sb = prepare_inputs(inputs, T)
    nc = build_program(T, sb)
    res = run_bass_kernel_spmd(nc, in_maps, list(range(NCORES)))
    return assemble(res.results, inputs, T)


# revision 6
# speedup vs baseline: 33.1788x; 33.1788x over previous
"""Trainium2 Bass kernel for the DVR-JANET recurrent cell.

Strategy (per spec sharding hint): data-parallel over batch across 8 cores
(8 sequences each), weights replicated.  Per core the T=1024 sequential
recurrence runs fully unrolled with all tensors in a transposed layout
[h on partitions, batch on free], so the 7 HxH matmuls per step use the
weights as PE-stationary operands (fp16, fast weight load) and the
activations as tiny moving operands.  sin/cos and all tanh-family gates are
evaluated by the Scalar engine out of one pinned activation-table set
(silu_and_others: sin + tanh) to avoid per-step table swaps; sigmoid is
rewritten as tanh.  Rank-1 input terms and all biases are folded into two
tiny block-diagonal matmuls.  The per-step elementwise chain is 5 fused
Vector-engine ops.  Final projections to I/Q run as a batched matmul pass
over the fp16 state history kept resident in SBUF.
"""

import functools
import numpy as np

import concourse.bacc as bacc
import concourse.mybir as mybir
from concourse import tile
import concourse.hw_specs as hw_specs
from concourse.bass_utils import run_bass_kernel_spmd

F32 = mybir.dt.float32
F16 = mybir.dt.float16
AF = mybir.ActivationFunctionType
OP = mybir.AluOpType

B, T, H = 64, 1024, 256
NCORES = 8
BL = B // NCORES          # batch per core = 8
CH = 128                  # XB chunk length (steps)
NCH = T // CH
NT = 32                   # full weight tiles

# ---------------------------------------------------------------------------
# Pin the ACT table set to silu_and_others (contains sin AND tanh) so the
# compiler never inserts per-step table swaps.
_orig_tables = hw_specs.get_activation_tables.__wrapped__


def _pinned_tables(arch):
    full = _orig_tables(arch)
    return {name: (funcs if name == "silu_and_others" else set())
            for name, funcs in full.items()}


def _pin_tables():
    fn = functools.cache(_pinned_tables)
    hw_specs.get_activation_tables = fn
    if hasattr(bacc, "get_activation_tables"):
        bacc.get_activation_tables = fn


# ---------------------------------------------------------------------------
_PROG_CACHE = {}


def build_program(Tn=T, sb=0.0, data_T=None):
    """Build the full 8-core SPMD program (same program, per-core inputs).

    data_T sizes the declared DRAM I/O (defaults to Tn); passing data_T >= Tn
    lets short-loop timing variants share input maps with the full build.
    """
    if data_T is None:
        data_T = Tn
    key = (Tn, float(sb), data_T)
    if key in _PROG_CACHE:
        return _PROG_CACHE[key]
    _pin_tables()
    nch = max(1, (data_T + CH - 1) // CH)
    nc = bacc.Bacc("TRN2", target_bir_lowering=False, debug=False,
                   num_devices=NCORES)

    # DRAM I/O (per core)
    w1_d = nc.dram_tensor("W1", [128, NT * 128], F16, kind="ExternalInput").ap()
    xwa_d = nc.dram_tensor("XWA", [5, 128], F16, kind="ExternalInput").ap()
    xwb_d = nc.dram_tensor("XWB", [2, 128], F16, kind="ExternalInput").ap()
    xwg_d = nc.dram_tensor("XWG", [6, 128], F16, kind="ExternalInput").ap()
    onesg_d = nc.dram_tensor("ONESG", [6, 48], F16, kind="ExternalInput").ap()
    xba_d = nc.dram_tensor("XBA", [nch, 5, CH * 32], F16, kind="ExternalInput").ap()
    xbb_d = nc.dram_tensor("XBB", [nch, 2, CH * 16], F16, kind="ExternalInput").ap()
    wp_d = nc.dram_tensor("WP", [128, 4], F16, kind="ExternalInput").ap()
    s0_d = nc.dram_tensor("S0", [128, 32], F16, kind="ExternalInput").ap()
    out_d = nc.dram_tensor("OUT", [1, 2 * data_T * BL], F32, kind="ExternalOutput").ap()

    with tile.TileContext(nc, trace_sim=False) as tc:
        with (
            tc.tile_pool(name="const", bufs=1) as cpool,
            tc.tile_pool(name="buf", bufs=1) as bufpool,
            tc.tile_pool(name="xba", bufs=2) as xbapool,
            tc.tile_pool(name="xbb", bufs=2) as xbbpool,
            tc.tile_pool(name="work", bufs=3) as wpool,
            tc.tile_pool(name="pa", bufs=2, space="PSUM") as ppa,
            tc.tile_pool(name="pb", bufs=2, space="PSUM") as ppb,
            tc.tile_pool(name="pg", bufs=2, space="PSUM") as ppg,
            tc.tile_pool(name="pp", bufs=2, space="PSUM") as ppp,
        ):
            wt1 = cpool.tile([128, NT * 128], F16, tag="wt1")
            xwa = cpool.tile([5, 128], F16, tag="xwa")
            xwb = cpool.tile([2, 128], F16, tag="xwb")
            xwg = cpool.tile([6, 128], F16, tag="xwg")
            onesg = cpool.tile([6, 48], F16, tag="onesg")
            wp = cpool.tile([128, 4], F16, tag="wp")
            # +2 slots: slot 0 = h0; slot t+1 = state after step t; one pad
            # slot so the projection's strided rhs windows stay in-bounds.
            buf = bufpool.tile([128, 32 * (Tn + 2)], F16, tag="buf")

            nc.sync.dma_start(wt1[:], w1_d)
            nc.sync.dma_start(xwa[:], xwa_d)
            nc.sync.dma_start(xwb[:], xwb_d)
            nc.sync.dma_start(xwg[:], xwg_d)
            nc.sync.dma_start(onesg[:], onesg_d)
            nc.sync.dma_start(wp[:], wp_d)
            nc.sync.dma_start(buf[:, 0:32], s0_d)

            def wtile(i):
                return wt1[:, 128 * i:128 * (i + 1)]

            xba_t = None
            xbb_t = None
            for t in range(Tn):
                s = t % CH
                if s == 0:
                    cc = t // CH
                    xba_t = xbapool.tile([5, CH * 32], F16, tag="xba")
                    xbb_t = xbbpool.tile([2, CH * 16], F16, tag="xbb")
                    nc.sync.dma_start(xba_t[:], xba_d[cc])
                    nc.sync.dma_start(xbb_t[:], xbb_d[cc])

                slot = buf[:, 32 * t:32 * (t + 1)]         # state entering step t
                nslot = buf[:, 32 * (t + 1):32 * (t + 2)]  # state after step t

                pa = ppa.tile([128, 32], F32, tag="pa")
                pb = ppb.tile([128, 16], F32, tag="pb")
                pg = ppg.tile([128, 48], F32, tag="pg")

                # --- gate bank pG: biases, then h-top contractions ---------
                nc.tensor.matmul(pg[:], xwg[:], onesg[:], start=True, stop=False)
                for j in (0, 1):
                    for k in (0, 1):
                        nc.tensor.matmul(pg[:, 8 * j:8 * j + 8],
                                         wtile(16 + 2 * j + k),
                                         slot[:, 8 * k:8 * k + 8],
                                         start=False, stop=False)
                        nc.tensor.matmul(pg[:, 16 + 8 * j:16 + 8 * j + 8],
                                         wtile(20 + 2 * j + k),
                                         slot[:, 16 + 8 * k:16 + 8 * k + 8],
                                         start=False, stop=False)

                # --- p = hI*hQ (fp16 for the PE) ---------------------------
                pt16 = wpool.tile([128, 16], F16, tag="pt16")
                nc.vector.tensor_mul(pt16[:], slot[:, 0:16], slot[:, 16:32])

                # --- theta / theta' bank pA --------------------------------
                nc.tensor.matmul(pa[:], xwa[:],
                                 xba_t[:, 32 * s:32 * s + 32],
                                 start=True, stop=False)
                for j in (0, 1):
                    for k in (0, 1):
                        nc.tensor.matmul(pa[:, 8 * j:8 * j + 8],
                                         wtile(2 * j + k),
                                         pt16[:, 8 * k:8 * k + 8],
                                         start=False, stop=(k == 1))
                        nc.tensor.matmul(pa[:, 16 + 8 * j:16 + 8 * j + 8],
                                         wtile(4 + 2 * j + k),
                                         pt16[:, 8 * k:8 * k + 8],
                                         start=False, stop=(k == 1))

                # --- a bank pB ---------------------------------------------
                nc.tensor.matmul(pb[:], xwb[:],
                                 xbb_t[:, 16 * s:16 * s + 16],
                                 start=True, stop=False)
                for j in (0, 1):
                    for k in (0, 1):
                        nc.tensor.matmul(pb[:, 8 * j:8 * j + 8],
                                         wtile(8 + 2 * j + k),
                                         pt16[:, 8 * k:8 * k + 8],
                                         start=False, stop=(k == 1))

                # --- f' into pG --------------------------------------------
                for j in (0, 1):
                    for k in (0, 1):
                        nc.tensor.matmul(pg[:, 32 + 8 * j:32 + 8 * j + 8],
                                         wtile(12 + 2 * j + k),
                                         pt16[:, 8 * k:8 * k + 8],
                                         start=False, stop=(k == 1))

                # --- SC = sin([theta | theta+pi/2]) = [sin | cos] ----------
                sct = wpool.tile([128, 32], F32, tag="sct")
                nc.scalar.activation(sct[:], pa[:], AF.Sin)

                # --- casa = (a + sb) * SC  -> [sa | ca] fp16 ---------------
                casa = wpool.tile([128, 32], F16, tag="casa")
                a_b = pb[:].rearrange("p (o f) -> p o f", o=1).broadcast_to([128, 2, 16])
                nc.vector.scalar_tensor_tensor(casa[:], a_b, float(sb), sct[:],
                                               OP.add, OP.mult)

                # --- bottom contractions into pG ---------------------------
                for j in (0, 1):
                    for k in (0, 1):
                        nc.tensor.matmul(pg[:, 8 * j:8 * j + 8],
                                         wtile(24 + 2 * j + k),
                                         casa[:, 16 + 8 * k:16 + 8 * k + 8],
                                         start=False, stop=(k == 1))
                        nc.tensor.matmul(pg[:, 16 + 8 * j:16 + 8 * j + 8],
                                         wtile(28 + 2 * j + k),
                                         casa[:, 8 * k:8 * k + 8],
                                         start=False, stop=(k == 1))

                # --- TH = tanh([gc | gs | f']) -----------------------------
                gbf = wpool.tile([128, 48], F32, tag="gbf")
                nc.scalar.activation(gbf[:], pg[:], AF.Tanh)

                # --- state update -----------------------------------------
                # S_new = 0.5*(1+tau)*(S - g) + g
                d = wpool.tile([128, 32], F32, tag="d")
                m2 = wpool.tile([128, 32], F32, tag="m2")
                nc.vector.tensor_tensor(d[:], slot[:], gbf[:, 0:32], OP.subtract)
                tau_b = gbf[:, 32:48].rearrange("p (o f) -> p o f", o=1) \
                                     .broadcast_to([128, 2, 16])
                nc.vector.scalar_tensor_tensor(m2[:], tau_b, 1.0, d[:],
                                               OP.add, OP.mult)
                nc.vector.scalar_tensor_tensor(nslot, m2[:], 0.5, gbf[:, 0:32],
                                               OP.mult, OP.add)

            # ----- projection: I/Q = WI.hI / WQ.hQ over all t --------------
            iqs = cpool.tile([1, 2 * Tn * BL], F32, tag="iqs")
            nchunk = (Tn + 63) // 64
            for c in range(nchunk):
                tc0 = 64 * c
                tlen = min(64, Tn - tc0)
                for q in (0, 1):
                    pp = ppp.tile([1, 512], F32, tag="pp")
                    for j in (0, 1):
                        st = 32 * (tc0 + 1) + 16 * q + 8 * j
                        rhs = buf[:, st:st + 32 * tlen] \
                            .rearrange("p (t b) -> p t b", t=tlen)[:, :, 0:8]
                        # rhs free AP: tlen steps of 32 cols, 8 batch cols
                        nc.tensor.matmul(pp[:, 0:8 * tlen], wp[:, 2 * q + j:2 * q + j + 1],
                                         rhs, start=(j == 0), stop=(j == 1))
                    dst = iqs[0:1, q * Tn * BL + BL * tc0:
                              q * Tn * BL + BL * (tc0 + tlen)]
                    if (c + q) % 2 == 0:
                        nc.scalar.copy(dst, pp[:, 0:8 * tlen])
                    else:
                        nc.vector.tensor_copy(dst, pp[:, 0:8 * tlen])
            nc.sync.dma_start(out_d[0:1, 0:2 * Tn * BL], iqs[:])

    nc.compile()
    _PROG_CACHE[key] = nc
    return nc


# ---------------------------------------------------------------------------
def prepare_inputs(inputs, Tn=T):
    """Host-side preprocessing: weight packing + per-core input maps."""
    f16 = np.float16
    x = np.asarray(inputs["x"], np.float32)
    hI0 = np.asarray(inputs["hI_0"], np.float32)[0]
    hQ0 = np.asarray(inputs["hQ_0"], np.float32)[0]
    c1 = float(np.asarray(inputs["c1"])[0])
    c2 = float(np.asarray(inputs["c2"])[0])
    c3 = float(np.asarray(inputs["c3"])[0])
    sc = c1 + c2 + c3
    sb = -(c1 / 3.0 + 2.0 * c2 / 3.0 + c3)
    Wa = np.asarray(inputs["Wa"], np.float32)[0]
    Wah = np.asarray(inputs["Wah"], np.float32)
    Wp1 = np.asarray(inputs["Wp1"], np.float32)[0]
    Wph = np.asarray(inputs["Wph"], np.float32)
    Wf = np.asarray(inputs["Wf"], np.float32)
    bf = np.asarray(inputs["bf"], np.float32)
    Wgc = np.asarray(inputs["Wgc"], np.float32)
    bgc = np.asarray(inputs["bgc"], np.float32)
    Wgs = np.asarray(inputs["Wgs"], np.float32)
    bgs = np.asarray(inputs["bgs"], np.float32)
    WI = np.asarray(inputs["WI"], np.float32)
    WQ = np.asarray(inputs["WQ"], np.float32)

    def tiles4(W):
        return [W[128 * k:128 * (k + 1), 128 * j:128 * (j + 1)]
                for j in (0, 1) for k in (0, 1)]

    tl = []
    tl += tiles4(Wph)               # 0-3 theta
    tl += tiles4(Wph)               # 4-7 theta'
    tl += tiles4(sc * Wah)          # 8-11 a (pre-scaled)
    tl += tiles4(0.5 * Wf)          # 12-15 f'
    tl += tiles4(Wgc[:H])           # 16-19 gc top
    tl += tiles4(Wgs[:H])           # 20-23 gs top
    tl += tiles4(Wgc[H:])           # 24-27 gc bot (ca)
    tl += tiles4(Wgs[H:])           # 28-31 gs bot (sa)
    W1 = np.concatenate(tl, axis=1).astype(f16)

    XWA = np.stack([Wp1[0:128], Wp1[128:256], Wp1[0:128], Wp1[128:256],
                    np.ones(128, np.float32)]).astype(f16)
    XWB = np.stack([(sc * Wa)[0:128], (sc * Wa)[128:256]]).astype(f16)
    XWG = np.stack([bgc[0:128], bgc[128:256], bgs[0:128], bgs[128:256],
                    0.5 * bf[0:128], 0.5 * bf[128:256]]).astype(f16)
    ONESG = np.zeros((6, 48), np.float32)
    for i in range(6):
        ONESG[i, 8 * i:8 * i + 8] = 1.0
    ONESG = ONESG.astype(f16)
    WP = np.stack([WI[0:128], WI[128:256], WQ[0:128], WQ[128:256]], axis=1).astype(f16)

    nch = max(1, (Tn + CH - 1) // CH)
    in_maps = []
    for c in range(NCORES):
        bs = slice(BL * c, BL * (c + 1))
        x0 = x[bs, :Tn, 0]            # [BL, Tn]
        x1 = x[bs, :Tn, 1]
        XBA = np.zeros((nch, 5, CH, 32), np.float32)
        XBB = np.zeros((nch, 2, CH, 16), np.float32)
        x1p = np.zeros((nch, CH, BL), np.float32)
        x0p = np.zeros((nch, CH, BL), np.float32)
        x1f = x1.T.reshape(-1, BL)    # [Tn, BL]
        x0f = x0.T.reshape(-1, BL)
        x1p.reshape(-1, BL)[:Tn] = x1f
        x0p.reshape(-1, BL)[:Tn] = x0f
        for r, cols in ((0, slice(0, 8)), (1, slice(8, 16)),
                        (2, slice(16, 24)), (3, slice(24, 32))):
            XBA[:, r, :, cols] = x1p
        XBA[:, 4, :, 16:32] = np.pi / 2
        XBB[:, 0, :, 0:8] = x0p
        XBB[:, 1, :, 8:16] = x0p
        S0 = np.zeros((128, 32), np.float32)
        for j in (0, 1):
            S0[:, 8 * j:8 * j + 8] = hI0[bs, 128 * j:128 * (j + 1)].T
            S0[:, 16 + 8 * j:24 + 8 * j] = hQ0[bs, 128 * j:128 * (j + 1)].T
        in_maps.append({
            "W1": W1, "XWA": XWA, "XWB": XWB, "XWG": XWG, "ONESG": ONESG,
            "WP": WP, "S0": S0.astype(f16),
            "XBA": XBA.reshape(nch, 5, CH * 32).astype(f16),
            "XBB": XBB.reshape(nch, 2, CH * 16).astype(f16),
        })
    return in_maps, sb


def assemble(results, inputs, Tn=T):
    bI = float(np.asarray(inputs["bI"])[0])
    bQ = float(np.asarray(inputs["bQ"])[0])
    out = np.zeros((B, Tn, 2), np.float32)
    nchunk = (Tn + 63) // 64
    for c in range(NCORES):
        arr = results[c]["OUT"].reshape(2, Tn * BL)
        for q in (0, 1):
            v = arr[q].reshape(nchunk, -1, BL)     # [chunk, t_in, b]
            v = v.transpose(2, 0, 1).reshape(BL, Tn)
            out[BL * c:BL * (c + 1), :, q] = v + (bI if q == 0 else bQ)
    return out


def kernel(**inputs) -> np.ndarray:
    in_maps, sb = prepare_inputs(inputs, T)
    nc = build_program(T, sb)
    res = run_bass_kernel_spmd(nc, in_maps, list(range(NCORES)))
    return assemble(res.results, inputs, T)


# revision 8
# speedup vs baseline: 68.1584x; 2.0543x over previous
"""Trainium2 Bass kernel for the DVR-JANET recurrent cell.

Strategy (per spec sharding hint): data-parallel over batch across 8 cores
(8 sequences each), weights replicated.  Per core the T=1024 sequential
recurrence runs fully unrolled with all tensors in a transposed layout
[h on partitions, batch on free], so the 7 HxH matmuls per step use the
weights as PE-stationary operands (fp16, fast weight load) and the
activations as tiny moving operands.  The 8 sequences are further split
into two half-batches of 4 whose step pipelines are interleaved half a
step out of phase, hiding the cross-engine semaphore latency of the
serial chain (p -> theta -> sin/cos -> ca/sa -> gates -> update).
sin/cos and all tanh-family gates are evaluated by the Scalar engine out
of one pinned activation-table set (silu_and_others: sin + tanh); sigmoid
is rewritten as tanh.  Rank-1 input terms and all biases are folded into
tiny block-diagonal matmuls.  Final projections to I/Q run as a batched
matmul pass over the fp16 state history kept resident in SBUF.
"""

import functools
import numpy as np

import concourse.bacc as bacc
import concourse.mybir as mybir
from concourse import tile
import concourse.hw_specs as hw_specs
from concourse.bass_utils import run_bass_kernel_spmd

F32 = mybir.dt.float32
F16 = mybir.dt.float16
AF = mybir.ActivationFunctionType
OP = mybir.AluOpType

B, T, H = 64, 1024, 256
NCORES = 8
BL = B // NCORES          # batch per core = 8
SUB = 2                   # staggered half-batches per core
BS = BL // SUB            # 4 sequences per half-batch
CH = 128                  # XB chunk length (steps)
NT = 32                   # full weight tiles

# ---------------------------------------------------------------------------
# Pin the ACT table set to silu_and_others (contains sin AND tanh) so the
# compiler never inserts per-step table swaps.
_orig_tables = hw_specs.get_activation_tables.__wrapped__


def _pinned_tables(arch):
    full = _orig_tables(arch)
    return {name: (funcs if name == "silu_and_others" else set())
            for name, funcs in full.items()}


def _pin_tables():
    fn = functools.cache(_pinned_tables)
    hw_specs.get_activation_tables = fn
    if hasattr(bacc, "get_activation_tables"):
        bacc.get_activation_tables = fn


# ---------------------------------------------------------------------------
_PROG_CACHE = {}


def build_program(Tn=T, sb=0.0, data_T=None):
    """Build the 8-core SPMD program.  data_T sizes declared DRAM I/O so
    short-loop timing variants can share input maps with the full build."""
    if data_T is None:
        data_T = Tn
    key = (Tn, float(sb), data_T)
    if key in _PROG_CACHE:
        return _PROG_CACHE[key]
    _pin_tables()
    nch = max(1, (data_T + CH - 1) // CH)
    nc = bacc.Bacc("TRN2", target_bir_lowering=False, debug=False,
                   num_devices=NCORES)

    w1_d = nc.dram_tensor("W1", [128, NT * 128], F16, kind="ExternalInput").ap()
    xwa_d = nc.dram_tensor("XWA", [5, 128], F16, kind="ExternalInput").ap()
    xwb_d = nc.dram_tensor("XWB", [2, 128], F16, kind="ExternalInput").ap()
    xwg_d = nc.dram_tensor("XWG", [6, 128], F16, kind="ExternalInput").ap()
    onesg_d = nc.dram_tensor("ONESG", [6, 24], F16, kind="ExternalInput").ap()
    xba_d = nc.dram_tensor("XBA", [nch, 5, CH * 32], F16, kind="ExternalInput").ap()
    xbb_d = nc.dram_tensor("XBB", [nch, 2, CH * 16], F16, kind="ExternalInput").ap()
    wp_d = nc.dram_tensor("WP", [128, 4], F16, kind="ExternalInput").ap()
    s0_d = nc.dram_tensor("S0", [128, 32], F16, kind="ExternalInput").ap()
    out_d = nc.dram_tensor("OUT", [1, 2 * data_T * BL], F32,
                           kind="ExternalOutput").ap()

    with tile.TileContext(nc, trace_sim=False) as tc:
        with (
            tc.tile_pool(name="const", bufs=1) as cpool,
            tc.tile_pool(name="buf", bufs=1) as bufpool,
            tc.tile_pool(name="xba", bufs=2) as xbapool,
            tc.tile_pool(name="xbb", bufs=2) as xbbpool,
            tc.tile_pool(name="work", bufs=3) as wpool,
            tc.tile_pool(name="pab0", bufs=2, space="PSUM") as ppab0,
            tc.tile_pool(name="pab1", bufs=2, space="PSUM") as ppab1,
            tc.tile_pool(name="pg0", bufs=2, space="PSUM") as ppg0,
            tc.tile_pool(name="pg1", bufs=2, space="PSUM") as ppg1,
        ):
            wt1 = cpool.tile([128, NT * 128], F16, tag="wt1")
            xwa = cpool.tile([5, 128], F16, tag="xwa")
            xwb = cpool.tile([2, 128], F16, tag="xwb")
            xwg = cpool.tile([6, 128], F16, tag="xwg")
            onesg = cpool.tile([6, 24], F16, tag="onesg")
            wp = cpool.tile([128, 4], F16, tag="wp")
            # slot t: state entering step t; 16 cols per half-batch
            # [hI-j0(4) hI-j1(4) hQ-j0(4) hQ-j1(4)]; +2 slots for h0 and the
            # projection's strided-window padding.
            buf = bufpool.tile([128, 32 * (Tn + 2)], F16, tag="buf")

            nc.sync.dma_start(wt1[:], w1_d)
            nc.sync.dma_start(xwa[:], xwa_d)
            nc.sync.dma_start(xwb[:], xwb_d)
            nc.sync.dma_start(xwg[:], xwg_d)
            nc.sync.dma_start(onesg[:], onesg_d)
            nc.sync.dma_start(wp[:], wp_d)
            nc.sync.dma_start(buf[:, 0:32], s0_d)

            def wtile(i):
                return wt1[:, 128 * i:128 * (i + 1)]

            pab_pool = (ppab0, ppab1)
            pg_pool = (ppg0, ppg1)
            chunk = {"xba": None, "xbb": None}
            # per-sub in-flight tiles across stages
            st = [dict() for _ in range(SUB)]

            def bcast2(ap, w):
                return ap.rearrange("p (o f) -> p o f", o=1).broadcast_to([128, 2, w])

            def s1(u, t):
                """pT + all layer-1 matmuls for half-batch u at step t."""
                s = t % CH
                if u == 0 and s == 0:
                    cc = t // CH
                    chunk["xba"] = xbapool.tile([5, CH * 32], F16, tag="xba", name="xbat")
                    chunk["xbb"] = xbbpool.tile([2, CH * 16], F16, tag="xbb", name="xbbt")
                    nc.sync.dma_start(chunk["xba"][:], xba_d[cc])
                    nc.sync.dma_start(chunk["xbb"][:], xbb_d[cc])
                slot = buf[:, 32 * t + 16 * u:32 * t + 16 * u + 16]
                pab = pab_pool[u].tile([128, 24], F32, tag="pab")
                pg = pg_pool[u].tile([128, 24], F32, tag="pg")
                pt = wpool.tile([128, 8], F16, tag=f"pt{u}")
                st[u].update(slot=slot, pab=pab, pg=pg, pt=pt, t=t)

                # gate bank: biases first (one start=True per bank)
                nc.tensor.matmul(pg[:], xwg[:], onesg[:], start=True, stop=False)
                for j in (0, 1):
                    for k in (0, 1):
                        nc.tensor.matmul(pg[:, 4 * j:4 * j + 4],
                                         wtile(16 + 2 * j + k),
                                         slot[:, 4 * k:4 * k + 4],
                                         start=False, stop=False)
                        nc.tensor.matmul(pg[:, 8 + 4 * j:8 + 4 * j + 4],
                                         wtile(20 + 2 * j + k),
                                         slot[:, 8 + 4 * k:8 + 4 * k + 4],
                                         start=False, stop=False)
                # p = hI*hQ
                nc.vector.tensor_mul(pt[:], slot[:, 0:8], slot[:, 8:16])
                # theta/theta'/a bank
                nc.tensor.matmul(pab[:, 0:16], xwa[:],
                                 chunk["xba"][:, 32 * s + 16 * u:32 * s + 16 * u + 16],
                                 start=True, stop=False)
                nc.tensor.matmul(pab[:, 16:24], xwb[:],
                                 chunk["xbb"][:, 16 * s + 8 * u:16 * s + 8 * u + 8],
                                 start=False, stop=False)
                for j in (0, 1):
                    for k in (0, 1):
                        nc.tensor.matmul(pab[:, 4 * j:4 * j + 4],
                                         wtile(2 * j + k),
                                         pt[:, 4 * k:4 * k + 4],
                                         start=False, stop=(k == 1))
                        nc.tensor.matmul(pab[:, 8 + 4 * j:8 + 4 * j + 4],
                                         wtile(4 + 2 * j + k),
                                         pt[:, 4 * k:4 * k + 4],
                                         start=False, stop=(k == 1))
                        nc.tensor.matmul(pab[:, 16 + 4 * j:16 + 4 * j + 4],
                                         wtile(8 + 2 * j + k),
                                         pt[:, 4 * k:4 * k + 4],
                                         start=False, stop=(k == 1))
                        nc.tensor.matmul(pg[:, 16 + 4 * j:16 + 4 * j + 4],
                                         wtile(12 + 2 * j + k),
                                         pt[:, 4 * k:4 * k + 4],
                                         start=False, stop=(k == 1))

            def s2(u):
                """sin/cos + ca/sa for half-batch u."""
                pab = st[u]["pab"]
                sc = wpool.tile([128, 16], F32, tag=f"sc{u}")
                casa = wpool.tile([128, 16], F16, tag=f"casa{u}")
                st[u].update(sc=sc, casa=casa)
                nc.scalar.activation(sc[:], pab[:, 0:16], AF.Sin)
                nc.vector.scalar_tensor_tensor(
                    casa[:], bcast2(pab[:, 16:24], 8), float(sb), sc[:],
                    OP.add, OP.mult)

            def s3(u):
                """bottom contractions + tanh for half-batch u."""
                pg = st[u]["pg"]
                casa = st[u]["casa"]
                for j in (0, 1):
                    for k in (0, 1):
                        nc.tensor.matmul(pg[:, 4 * j:4 * j + 4],
                                         wtile(24 + 2 * j + k),
                                         casa[:, 8 + 4 * k:8 + 4 * k + 4],
                                         start=False, stop=(k == 1))
                        nc.tensor.matmul(pg[:, 8 + 4 * j:8 + 4 * j + 4],
                                         wtile(28 + 2 * j + k),
                                         casa[:, 4 * k:4 * k + 4],
                                         start=False, stop=(k == 1))
                gbf = wpool.tile([128, 24], F32, tag=f"gbf{u}")
                st[u]["gbf"] = gbf
                nc.scalar.activation(gbf[:], pg[:], AF.Tanh)

            def s4(u):
                """state update for half-batch u."""
                t = st[u]["t"]
                slot = st[u]["slot"]
                gbf = st[u]["gbf"]
                nslot = buf[:, 32 * (t + 1) + 16 * u:32 * (t + 1) + 16 * u + 16]
                d = wpool.tile([128, 16], F32, tag=f"d{u}")
                m2 = wpool.tile([128, 16], F32, tag=f"m2{u}")
                nc.vector.tensor_tensor(d[:], slot[:], gbf[:, 0:16], OP.subtract)
                nc.vector.scalar_tensor_tensor(m2[:], bcast2(gbf[:, 16:24], 8),
                                               1.0, d[:], OP.add, OP.mult)
                nc.vector.scalar_tensor_tensor(nslot, m2[:], 0.5, gbf[:, 0:16],
                                               OP.mult, OP.add)

            # --- staggered emission: V runs half a step behind U -----------
            for t in range(Tn):
                s1(0, t)
                if t > 0:
                    s3(1)
                s2(0)
                if t > 0:
                    s4(1)
                s3(0)
                s1(1, t)
                s4(0)
                s2(1)
            s3(1)
            s4(1)

            # ----- projection: I/Q = WI.hI / WQ.hQ over all t --------------
            iqs = cpool.tile([1, 2 * Tn * BL], F32, tag="iqs")
            nchunk = (Tn + CH - 1) // CH
            for c in range(nchunk):
                tc0 = CH * c
                tlen = min(CH, Tn - tc0)
                for q in (0, 1):
                    for u in range(SUB):
                        pp = pg_pool[u].tile([1, 512], F32, tag="pg")
                        for j in (0, 1):
                            stc = 32 * (tc0 + 1) + 16 * u + 8 * q + 4 * j
                            rhs = buf[:, stc:stc + 32 * tlen] \
                                .rearrange("p (t b) -> p t b", t=tlen)[:, :, 0:4]
                            nc.tensor.matmul(pp[:, 0:4 * tlen],
                                             wp[:, 2 * q + j:2 * q + j + 1],
                                             rhs, start=(j == 0), stop=(j == 1))
                        dst = iqs[0:1, q * Tn * BL + u * Tn * BS + BS * tc0:
                                  q * Tn * BL + u * Tn * BS + BS * (tc0 + tlen)]
                        if (c + q + u) % 2 == 0:
                            nc.scalar.copy(dst, pp[:, 0:4 * tlen])
                        else:
                            nc.vector.tensor_copy(dst, pp[:, 0:4 * tlen])
            nc.sync.dma_start(out_d[0:1, 0:2 * Tn * BL], iqs[:])

    nc.compile()
    _PROG_CACHE[key] = nc
    return nc


# ---------------------------------------------------------------------------
def prepare_inputs(inputs, Tn=T):
    """Host-side preprocessing: weight packing + per-core input maps."""
    f16 = np.float16
    x = np.asarray(inputs["x"], np.float32)
    hI0 = np.asarray(inputs["hI_0"], np.float32)[0]
    hQ0 = np.asarray(inputs["hQ_0"], np.float32)[0]
    c1 = float(np.asarray(inputs["c1"])[0])
    c2 = float(np.asarray(inputs["c2"])[0])
    c3 = float(np.asarray(inputs["c3"])[0])
    sc = c1 + c2 + c3
    sb = -(c1 / 3.0 + 2.0 * c2 / 3.0 + c3)
    Wa = np.asarray(inputs["Wa"], np.float32)[0]
    Wah = np.asarray(inputs["Wah"], np.float32)
    Wp1 = np.asarray(inputs["Wp1"], np.float32)[0]
    Wph = np.asarray(inputs["Wph"], np.float32)
    Wf = np.asarray(inputs["Wf"], np.float32)
    bf = np.asarray(inputs["bf"], np.float32)
    Wgc = np.asarray(inputs["Wgc"], np.float32)
    bgc = np.asarray(inputs["bgc"], np.float32)
    Wgs = np.asarray(inputs["Wgs"], np.float32)
    bgs = np.asarray(inputs["bgs"], np.float32)
    WI = np.asarray(inputs["WI"], np.float32)
    WQ = np.asarray(inputs["WQ"], np.float32)

    def tiles4(W):
        return [W[128 * k:128 * (k + 1), 128 * j:128 * (j + 1)]
                for j in (0, 1) for k in (0, 1)]

    tl = []
    tl += tiles4(Wph)               # 0-3 theta
    tl += tiles4(Wph)               # 4-7 theta'
    tl += tiles4(sc * Wah)          # 8-11 a (pre-scaled)
    tl += tiles4(0.5 * Wf)          # 12-15 f'
    tl += tiles4(Wgc[:H])           # 16-19 gc top
    tl += tiles4(Wgs[:H])           # 20-23 gs top
    tl += tiles4(Wgc[H:])           # 24-27 gc bot (ca)
    tl += tiles4(Wgs[H:])           # 28-31 gs bot (sa)
    W1 = np.concatenate(tl, axis=1).astype(f16)

    XWA = np.stack([Wp1[0:128], Wp1[128:256], Wp1[0:128], Wp1[128:256],
                    np.ones(128, np.float32)]).astype(f16)
    XWB = np.stack([(sc * Wa)[0:128], (sc * Wa)[128:256]]).astype(f16)
    XWG = np.stack([bgc[0:128], bgc[128:256], bgs[0:128], bgs[128:256],
                    0.5 * bf[0:128], 0.5 * bf[128:256]]).astype(f16)
    ONESG = np.zeros((6, 24), np.float32)
    for i in range(6):
        ONESG[i, 4 * i:4 * i + 4] = 1.0
    ONESG = ONESG.astype(f16)
    WP = np.stack([WI[0:128], WI[128:256], WQ[0:128], WQ[128:256]],
                  axis=1).astype(f16)

    nch = max(1, (Tn + CH - 1) // CH)
    in_maps = []
    for c in range(NCORES):
        XBA = np.zeros((nch, 5, CH, 32), np.float32)
        XBB = np.zeros((nch, 2, CH, 16), np.float32)
        for u in range(SUB):
            bs = slice(BL * c + BS * u, BL * c + BS * (u + 1))
            x1p = np.zeros((nch, CH, BS), np.float32)
            x0p = np.zeros((nch, CH, BS), np.float32)
            x1p.reshape(-1, BS)[:Tn] = x[bs, :Tn, 1].T
            x0p.reshape(-1, BS)[:Tn] = x[bs, :Tn, 0].T
            for r, c0 in ((0, 0), (1, 4), (2, 8), (3, 12)):
                XBA[:, r, :, 16 * u + c0:16 * u + c0 + 4] = x1p
            XBA[:, 4, :, 16 * u + 8:16 * u + 16] = np.pi / 2
            XBB[:, 0, :, 8 * u:8 * u + 4] = x0p
            XBB[:, 1, :, 8 * u + 4:8 * u + 8] = x0p
        S0 = np.zeros((128, 32), np.float32)
        for u in range(SUB):
            bs = slice(BL * c + BS * u, BL * c + BS * (u + 1))
            for j in (0, 1):
                S0[:, 16 * u + 4 * j:16 * u + 4 * j + 4] = \
                    hI0[bs, 128 * j:128 * (j + 1)].T
                S0[:, 16 * u + 8 + 4 * j:16 * u + 12 + 4 * j] = \
                    hQ0[bs, 128 * j:128 * (j + 1)].T
        in_maps.append({
            "W1": W1, "XWA": XWA, "XWB": XWB, "XWG": XWG, "ONESG": ONESG,
            "WP": WP, "S0": S0.astype(f16),
            "XBA": XBA.reshape(nch, 5, CH * 32).astype(f16),
            "XBB": XBB.reshape(nch, 2, CH * 16).astype(f16),
        })
    return in_maps, sb


def assemble(results, inputs, Tn=T):
    bI = float(np.asarray(inputs["bI"])[0])
    bQ = float(np.asarray(inputs["bQ"])[0])
    out = np.zeros((B, Tn, 2), np.float32)
    for c in range(NCORES):
        arr = results[c]["OUT"].reshape(-1)[:2 * Tn * BL]
        for q in (0, 1):
            for u in range(SUB):
                seg = arr[q * Tn * BL + u * Tn * BS:
                          q * Tn * BL + (u + 1) * Tn * BS]
                v = seg.reshape(-1, BS)        # [t, b4] over chunk-major t
                nchunk = (Tn + CH - 1) // CH
                v = v.reshape(nchunk, -1, BS)  # [chunk, t_in, b4]
                v = v.transpose(2, 0, 1).reshape(BS, Tn)
                rows = slice(BL * c + BS * u, BL * c + BS * (u + 1))
                out[rows, :, q] = v + (bI if q == 0 else bQ)
    return out


def kernel(**inputs) -> np.ndarray:
    in_maps, sb = prepare_inputs(inputs, T)
    nc = build_program(T, sb)
    res = run_bass_kernel_spmd(nc, in_maps, list(range(NCORES)))
    return assemble(res.results, inputs, T)
